# revision 14
# baseline (speedup 1.0000x reference)
"""Attention (softmax over the QUERY axis) on 8 trn2 NeuronCores.

Problem: x:[4,2048,1024], Wq/Wk/Wv:[1024,1024], bq/bk/bv:[1024]
    q = x@Wq+bq ; k = x@Wk+bk ; v = x@Wv+bv
    scores = einsum('bqd,bkd->bqk', q, k) / 32
    attn   = softmax(scores, axis=1)          # over q (dim 1)!
    out    = einsum('bqk,bkv->bqv', attn, v)

Sharding: 4 batches x 2-way split of the QUERY axis across 8 cores
(core c -> batch c//2, query-half c%2).  The token range is split the
same way, so each core projects K and V only for ITS half of the
tokens (no redundant projection work); the halves are exchanged with
two pairwise 2MB AllGathers (K^T, then V) that hide behind the Q
projection and score phases.  The softmax denominator D[k] =
sum_q exp(s[q,k]) needs both query halves: two 4KB pairwise
AllReduces (one per token half, pipelined so the attn*V phase can
start on the first half while the second reduces).

All matmul inputs are bf16 (PSUM accumulates fp32); simulated rms rel
err 4.8e-3 vs the 2e-2 gate.  bf16 also enables the compiler's fast
weight load (FWL is disabled for fp32 stationaries) and halves DMA
and SBUF footprints.  Per-core matmul work: 896 128x128x512 MMs.

Queue discipline (engine queues are in-order, so a waiting
instruction blocks everything behind it):
  sync   - x prefetch (even et), V staging out, K^T + V loads back
           from the AllGathers, half the y writes
  scalar - x prefetch (odd et), biases/Wv prefetch, all PSUM->SBUF
           activations/copies, AllReduce input staging, half y writes
  gpsimd - Wk/Wq prefetch, K^T staging out, all collectives,
           denominator 1 reciprocal + first-half PT scaling
  vector - V bias adds, denominator row-sums, denominator 2
           reciprocal + second-half PT scaling
"""

import sys

if "/opt/trn_rl_repo" not in sys.path:
    sys.path.insert(0, "/opt/trn_rl_repo")

import numpy as np

P = 128  # SBUF partitions


class Cfg:
    def __init__(self, B=4, S=2048, E=1024, D=1024, NB=512, n_cores=8):
        self.B, self.S, self.E, self.D, self.NB = B, S, E, D, NB
        self.SH = S // 2          # per-core query-half == token-half length
        self.NE = E // P          # e (contraction) tiles
        self.ND = D // P          # d tiles
        self.NQb = self.SH // NB  # q 512-blocks (my half)
        self.NKbL = self.SH // NB # local-token 512-blocks
        self.NKt = S // P         # k 128-tiles (full)
        self.NKtH = self.NKt // 2 # k 128-tiles per half
        self.NQt = self.SH // P   # q 128-tiles (my half)
        self.NDVB = D // NB       # dv 512-blocks
        self.n_cores = n_cores
        self.groups = [[2 * i, 2 * i + 1] for i in range(n_cores // 2)]


PROD = Cfg()


def build_nc(cfg: Cfg):
    from concourse import bacc, bass, mybir, tile

    f32 = mybir.dt.float32
    bf16 = mybir.dt.bfloat16
    AF = mybir.ActivationFunctionType
    X = mybir.AxisListType.X
    ts = bass.ts

    B, S, E, D, NB = cfg.B, cfg.S, cfg.E, cfg.D, cfg.NB
    SH, NE, ND = cfg.SH, cfg.NE, cfg.ND
    NQb, NKbL, NKt, NKtH = cfg.NQb, cfg.NKbL, cfg.NKt, cfg.NKtH
    NQt, NDVB = cfg.NQt, cfg.NDVB
    KT_PER_B = NB // P
    inv_sqrt_d = 1.0 / float(np.sqrt(np.float32(D)))

    nc = bacc.Bacc(None, num_devices=cfg.n_cores)

    # Per-core inputs (host pre-shards / pre-transposes / casts to bf16).
    # xth: X^T columns of MY token-half (== my query-half).
    xth_d = nc.declare_dram_parameter("xth", [E, SH], bf16, isOutput=False)
    wq_d = nc.declare_dram_parameter("wq", [E, D], bf16, isOutput=False)
    wk_d = nc.declare_dram_parameter("wk", [E, D], bf16, isOutput=False)
    wv_d = nc.declare_dram_parameter("wv", [E, D], bf16, isOutput=False)
    # bk and bq packed per-dk-tile into one [P, 2*ND] tensor (col dt = bk
    # tile dt, col ND+dt = bq tile dt) so startup is a single DMA trigger.
    bqk_d = nc.declare_dram_parameter("bqk", [P, 2 * ND], f32, isOutput=False)
    bvb_d = nc.declare_dram_parameter("bvb", [P, D], f32, isOutput=False)
    y_d = nc.declare_dram_parameter("y", [SH, D], f32, isOutput=True)

    with tile.TileContext(nc) as tc:
        with (
            tc.tile_pool(name="w", bufs=16) as w_pool,
            tc.tile_pool(name="x", bufs=8) as x_pool,
            tc.tile_pool(name="kw", bufs=4) as kw_pool,
            tc.tile_pool(name="qt", bufs=1) as qt_pool,
            tc.tile_pool(name="ktw", bufs=1) as ktw_pool,
            tc.tile_pool(name="v", bufs=16) as v_pool,
            tc.tile_pool(name="pt", bufs=1) as pt_pool,
            tc.tile_pool(name="ob", bufs=2) as out_pool,
            tc.tile_pool(name="small", bufs=1) as small_pool,
            tc.tile_pool(name="ps", bufs=8, space="PSUM") as ps_pool,
            tc.tile_pool(name="dram", bufs=1, space="DRAM") as dram_pool,
        ):
            # DRAM staging for the collectives (rank-major AllGather outputs).
            kag_in = dram_pool.tile([D, SH], bf16, tag="kag_in")
            kag_out = dram_pool.tile([2 * D, SH], bf16, tag="kag_out")
            vag_in = dram_pool.tile([SH, D], bf16, tag="vag_in")
            vag_out = dram_pool.tile([S, D], bf16, tag="vag_out")
            ar1_in = dram_pool.tile([P, NKtH], f32, tag="ar1_in")
            ar1_out = dram_pool.tile([P, NKtH], f32, tag="ar1_out")
            ar2_in = dram_pool.tile([P, NKtH], f32, tag="ar2_in")
            ar2_out = dram_pool.tile([P, NKtH], f32, tag="ar2_out")
            # ---- PE warmup: dummy bf16 matmuls flip the HAM clock gate
            # to 8/8 while the first input DMAs are still in flight ----
            wu_a = small_pool.tile([P, P], bf16, tag="wua")
            nc.vector.memset(wu_a[:], 0.0)
            wu_b = small_pool.tile([P, P], bf16, tag="wub")
            nc.vector.memset(wu_b[:], 0.0)
            wu_ps = ps_pool.tile([P, NB], f32, tag="ps", name="wups")
            for _ in range(48):
                nc.tensor.matmul(wu_ps[:, :P], wu_a[:], wu_b[:], start=True, stop=True)

            # ---- prefetch: Wk first (startup critical path), x, biases, Wv
            wk_t = []
            for et in range(NE):
                w = w_pool.tile([P, D], bf16, tag="w", name=f"wk{et}")
                nc.gpsimd.dma_start(w[:], wk_d[ts(et, P), :])
                wk_t.append(w)
            xt = []
            for et in range(NE):
                t = x_pool.tile([P, SH], bf16, tag="x", name=f"x{et}")
                eng = nc.sync if et % 2 == 0 else nc.scalar
                eng.dma_start(t[:], xth_d[ts(et, P), :])
                xt.append(t)
            bqk_t = small_pool.tile([P, 2 * ND], f32, tag="bqk")
            nc.scalar.dma_start(bqk_t[:], bqk_d[:])
            bk_t = [bqk_t[:, dt:dt + 1] for dt in range(ND)]
            bq_t = [bqk_t[:, ND + dt:ND + dt + 1] for dt in range(ND)]
            bvb_t = small_pool.tile([P, D], f32, tag="bvb")
            nc.scalar.dma_start(bvb_t[:], bvb_d[:])
            wv_t = []
            for et in range(NE):
                w = w_pool.tile([P, D], bf16, tag="w", name=f"wv{et}")
                nc.scalar.dma_start(w[:], wv_d[ts(et, P), :])
                wv_t.append(w)

            # ---- phase K: project MY token-half, stage K^T to DRAM ----
            for kb in range(NKbL):
                for dt in range(ND):
                    ps = ps_pool.tile([P, NB], f32, tag="ps", name="psk")
                    for et in range(NE):
                        nc.tensor.matmul(
                            ps[:], wk_t[et][:, ts(dt, P)], xt[et][:, ts(kb, NB)],
                            start=(et == 0), stop=(et == NE - 1),
                        )
                    kw = kw_pool.tile([P, NB], bf16, tag="kw", name=f"kw{kb}_{dt}")
                    nc.scalar.activation(kw[:], ps[:], AF.Identity, bias=bk_t[dt][:])
                    nc.gpsimd.dma_start(kag_in[ts(dt, P), ts(kb, NB)], kw[:])

            nc.gpsimd.collective_compute(
                "AllGather",
                mybir.AluOpType.bypass,
                replica_groups=cfg.groups,
                ins=[kag_in[:].opt()],
                outs=[kag_out[:].opt()],
            )

            # Wq prefetch sits behind AG-K on the gpsimd queue so it cannot
            # delay the collective; its slots reuse Wk's (dead after phase K).
            wq_t = []
            for et in range(NE):
                w = w_pool.tile([P, D], bf16, tag="w", name=f"wq{et}")
                nc.gpsimd.dma_start(w[:], wq_d[ts(et, P), :])
                wq_t.append(w)

            # ---- phase V: project MY token-half, stage V to DRAM ----
            for kb in range(NKbL):
                for kt4 in range(KT_PER_B):
                    i = kb * KT_PER_B + kt4
                    vl = v_pool.tile([P, D], bf16, tag="v", name=f"vloc{i}")
                    for dvb in range(NDVB):
                        ps = ps_pool.tile([P, NB], f32, tag="ps", name="psv")
                        for et in range(NE):
                            nc.tensor.matmul(
                                ps[:], xt[et][:, kb * NB + kt4 * P:kb * NB + (kt4 + 1) * P],
                                wv_t[et][:, ts(dvb, NB)],
                                start=(et == 0), stop=(et == NE - 1),
                            )
                        nc.vector.tensor_add(
                            vl[:, ts(dvb, NB)], ps[:], bvb_t[:, ts(dvb, NB)]
                        )
                    nc.sync.dma_start(vag_in[ts(i, P), :], vl[:])

            nc.gpsimd.collective_compute(
                "AllGather",
                mybir.AluOpType.bypass,
                replica_groups=cfg.groups,
                ins=[vag_in[:].opt()],
                outs=[vag_out[:].opt()],
            )

            # ---- phase Q: QT[dt][d, q] = Q[q, d]^T for MY q-half (resident) ----
            qt_tiles = []
            for dt in range(ND):
                q = qt_pool.tile([P, SH], bf16, tag=f"qt{dt}", name=f"qtt{dt}")
                qt_tiles.append(q)
            for qb in range(NQb):
                for dt in range(ND):
                    ps = ps_pool.tile([P, NB], f32, tag="ps", name="psq")
                    for et in range(NE):
                        nc.tensor.matmul(
                            ps[:], wq_t[et][:, ts(dt, P)], xt[et][:, ts(qb, NB)],
                            start=(et == 0), stop=(et == NE - 1),
                        )
                    nc.scalar.activation(
                        qt_tiles[dt][:, ts(qb, NB)], ps[:], AF.Identity, bias=bq_t[dt][:]
                    )

            # ---- K^T (both halves, global token order) back from the AllGather ----
            ktw = {}
            for g in range(2):
                for dt in range(ND):
                    t = ktw_pool.tile([P, SH], bf16, tag=f"ktw{g}_{dt}", name=f"ktw{g}_{dt}")
                    nc.sync.dma_start(t[:], kag_out[g * D + dt * P:g * D + (dt + 1) * P, :])
                    ktw[(g, dt)] = t

            # ---- V tiles (both halves, global token order) from the AllGather.
            # On the sync queue after the K^T loads: they only fire once AG-V
            # completes, well before the attn*V phase needs them. ----
            v_tiles = []
            for kt in range(NKt):
                vt = v_pool.tile([P, D], bf16, tag="v", name=f"vt{kt}")
                nc.sync.dma_start(vt[:], vag_out[ts(kt, P), :])
                v_tiles.append(vt)

            # ---- scores: PT[kt][k, q] = exp(s[q, k]/32), rs = partial denoms ----
            pt_tiles = []
            rs_t = []
            for kt in range(NKt):
                ptt = pt_pool.tile([P, SH], bf16, tag=f"pt{kt}", name=f"ptt{kt}")
                pt_tiles.append(ptt)
                rst = small_pool.tile([P, NQb], f32, tag=f"rs{kt}", name=f"rst{kt}")
                rs_t.append(rst)
            rs_h = []
            for g in range(2):
                rsh = small_pool.tile([P, NKtH], f32, tag=f"rsh{g}")
                rs_h.append(rsh)

            for g in range(2):
                for j in range(NKtH):
                    kt = g * NKtH + j
                    for qb in range(NQb):
                        ps = ps_pool.tile([P, NB], f32, tag="ps", name="pss")
                        for dt in range(ND):
                            nc.tensor.matmul(
                                ps[:], ktw[(g, dt)][:, ts(j, P)],
                                qt_tiles[dt][:, ts(qb, NB)],
                                start=(dt == 0), stop=(dt == ND - 1),
                            )
                        nc.scalar.activation(
                            pt_tiles[kt][:, ts(qb, NB)], ps[:], AF.Exp,
                            scale=inv_sqrt_d,
                            accum_out=rs_t[kt][:, qb:qb + 1],
                        )
                    nc.vector.reduce_sum(rs_h[g][:, j:j + 1], rs_t[kt][:], axis=X)
                if g == 0:
                    # first-half denominators: AllReduce + scale PT[0:8] on
                    # gpsimd while the second half is still scoring
                    nc.scalar.dma_start(ar1_in[:], rs_h[0][:])
                    nc.gpsimd.collective_compute(
                        "AllReduce",
                        mybir.AluOpType.add,
                        replica_groups=cfg.groups,
                        ins=[ar1_in[:].opt()],
                        outs=[ar1_out[:].opt()],
                    )
                    rsum1 = small_pool.tile([P, NKtH], f32, tag="rsum1")
                    nc.gpsimd.dma_start(rsum1[:], ar1_out[:])
                    rcp1 = small_pool.tile([P, NKtH], f32, tag="rcp1")
                    nc.vector.reciprocal(rcp1[:], rsum1[:])
                    for j in range(NKtH):
                        nc.vector.tensor_scalar_mul(
                            pt_tiles[j][:], pt_tiles[j][:], rcp1[:, j:j + 1]
                        )

            # second-half denominators: AllReduce staged from scalar, scaled
            # on vector (idle once the row-sums finish)
            nc.scalar.dma_start(ar2_in[:], rs_h[1][:])
            nc.gpsimd.collective_compute(
                "AllReduce",
                mybir.AluOpType.add,
                replica_groups=cfg.groups,
                ins=[ar2_in[:].opt()],
                outs=[ar2_out[:].opt()],
            )
            rsum2 = small_pool.tile([P, NKtH], f32, tag="rsum2")
            nc.scalar.dma_start(rsum2[:], ar2_out[:])
            rcp2 = small_pool.tile([P, NKtH], f32, tag="rcp2")
            nc.vector.reciprocal(rcp2[:], rsum2[:])
            for j in range(NKtH):
                nc.vector.tensor_scalar_mul(
                    pt_tiles[NKtH + j][:], pt_tiles[NKtH + j][:], rcp2[:, j:j + 1]
                )

            # ---- phase AV: y[q, dv] = sum_k attn[k,v] * V[k,dv] ----
            # Two groups of 8 PSUM tiles.  Within a group all half-0 kt
            # matmuls issue first (PT[0:8] is scaled while half 1's
            # denominators reduce); the half-1 pass drains each tile to
            # DRAM as soon as its accumulation stops.
            for grp in range(2):
                qts = range(grp * NQt // 2, (grp + 1) * NQt // 2)
                pss = {}
                for qt in qts:
                    for dvb in range(NDVB):
                        pss[(qt, dvb)] = ps_pool.tile([P, NB], f32, tag="ps", name="psav")
                for qt in qts:
                    for dvb in range(NDVB):
                        for j in range(NKtH):
                            nc.tensor.matmul(
                                pss[(qt, dvb)][:], pt_tiles[j][:, ts(qt, P)],
                                v_tiles[j][:, ts(dvb, NB)],
                                start=(j == 0), stop=False,
                            )
                for qt in qts:
                    for dvb in range(NDVB):
                        for j in range(NKtH):
                            kt = NKtH + j
                            nc.tensor.matmul(
                                pss[(qt, dvb)][:], pt_tiles[kt][:, ts(qt, P)],
                                v_tiles[kt][:, ts(dvb, NB)],
                                start=False, stop=(j == NKtH - 1),
                            )
                        ob = out_pool.tile([P, NB], f32, tag="ob", name="ob")
                        nc.scalar.copy(ob[:], pss[(qt, dvb)][:])
                        eng = nc.sync if (qt + dvb) % 2 == 0 else nc.scalar
                        eng.dma_start(y_d[ts(qt, P), ts(dvb, NB)], ob[:])

    nc.compile()
    return nc


def make_in_maps(cfg: Cfg, x, Wq, bq, Wk, bk, Wv, bv):
    import ml_dtypes

    SH = cfg.SH
    f32 = np.float32
    bf = ml_dtypes.bfloat16
    in_maps = []
    shared = {
        "wq": np.ascontiguousarray(Wq, dtype=bf),
        "wk": np.ascontiguousarray(Wk, dtype=bf),
        "wv": np.ascontiguousarray(Wv, dtype=bf),
        "bqk": np.ascontiguousarray(
            np.concatenate(
                [np.reshape(bk, (-1, P)).T, np.reshape(bq, (-1, P)).T], axis=1
            ),
            dtype=f32,
        ),
        "bvb": np.ascontiguousarray(
            np.broadcast_to(np.reshape(bv, (1, -1)), (P, len(np.ravel(bv)))), dtype=f32
        ),
    }
    for c in range(cfg.n_cores):
        b, h = c // 2, c % 2
        xb = np.asarray(x[b], dtype=f32)
        m = dict(shared)
        m["xth"] = np.ascontiguousarray(xb[h * SH:(h + 1) * SH, :].T, dtype=bf)
        in_maps.append(m)
    return in_maps


def run(inputs: dict, cfg: Cfg = PROD, trace: bool = False):
    from concourse.bass_utils import run_bass_kernel_spmd

    nc = build_nc(cfg)
    in_maps = make_in_maps(cfg, inputs["x"], inputs["Wq"], inputs["bq"],
                           inputs["Wk"], inputs["bk"], inputs["Wv"], inputs["bv"])
    res = run_bass_kernel_spmd(nc, in_maps, list(range(cfg.n_cores)), trace=trace)
    out = assemble(cfg, [r["y"] for r in res.results])
    return out, res


def assemble(cfg: Cfg, ys):
    """Core 2b holds q-rows [0, S/2), core 2b+1 holds [S/2, S) of batch b."""
    B, S, D = cfg.B, cfg.S, cfg.D
    out = np.empty((B, S, D), dtype=np.float32)
    for b in range(B):
        out[b, : cfg.SH] = ys[2 * b]
        out[b, cfg.SH:] = ys[2 * b + 1]
    return out


def kernel(**inputs) -> np.ndarray:
    out, _ = run(inputs, PROD, trace=False)
    return out
